# revision 23
# baseline (speedup 1.0000x reference)
"""Expert-LoRA routed delta kernel for Trainium2 (8 NeuronCores).

Math (per batch b, with routing resolved on host):
    out[b] = base[b] + x[b] @ At_b @ Bwt_b
where
    At_b  [H, 32] = concat_k A_{e_k}^T              (e_k = top_k_indices[b, k])
    Bwt_b [32, H] = concat_k (w_{b,k} * scaling * B_{e_k}^T)

Host-side prep folds everything cheap into input layout:
  * expert gather + gate weights + lora scaling -> tiny At/Bwt tables;
  * x is pre-transposed to an h-major tiled layout (xt[half, j, p, s]) so the
    tensor engine can contract over H without any on-chip transposes, and
    each DMA reads one fully contiguous block.

Device pipeline per core (= one batch; B == n_cores == 8):
  for each 512-row S-macro: load xT halves -> 28 accumulating matmuls
  (rank-32 down-projection, N=512) -> per 128-row block: 7 up-projection
  matmuls (K=32, N=512) + vector add with base -> store. Pure DMA-bound:
  every compute engine sits far below the ~250us/core HBM roofline.

Sharding: data-parallel over batch (spec sharding_hint), SPMD program.
"""

import sys

if "/opt/trn_rl_repo" not in sys.path:
    sys.path.insert(0, "/opt/trn_rl_repo")

import numpy as np

# Problem shape (hardcoded per contract; must match setup_inputs()).
B, S, H = 8, 2048, 3584
E, R, TOPK = 8, 16, 2
KR = TOPK * R  # 32 = concatenated rank
SCALING = 32.0 / 16.0
N_CORES = 8

S_BLK = 128
NS = S // S_BLK  # 16 s-blocks
HB = H // 128  # 28 h-blocks of 128
HC = H // 512  # 7 h-chunks of 512
NMAC = S // 512  # 4 S-macros of 512 rows
HHALF = HB // 2  # 14 h-blocks per xT half-tile

_CACHE: dict = {}


def _split_sync_waits(nc, max_waits=1):
    """This walrus build rejects >max_waits sync-wait commands on a single
    instruction (setupSyncWait: 'Too many sync wait commands'). Hoist excess
    waits onto same-engine NOPs inserted immediately before the instruction.
    Same-queue ordering makes this equivalent: the engine blocks on each
    hoisted wait before reaching the original instruction. Monotonic (ge)
    waits are hoisted first; eq-waits stay on the instruction when possible.
    """
    import concourse.mybir as mybir

    for fn in nc.m.functions:
        for bb in fn.blocks:
            new_insts = []
            for inst in bb.instructions:
                si = inst.sync_info
                if si is not None and si.on_wait and len(si.on_wait) > max_waits:
                    waits = list(si.on_wait)
                    ge = [w for w in waits if w.wait_mode != "sem-eq-imm"]
                    eq = [w for w in waits if w.wait_mode == "sem-eq-imm"]
                    keep = (eq + ge)[-max_waits:]
                    hoist = (eq + ge)[:-max_waits]
                    for w in hoist:
                        new_insts.append(
                            mybir.InstNoOp(
                                name=f"I-{nc.next_id()}",
                                engine=inst.engine,
                                bass_nofuse=True,
                                sync_info=mybir.SyncInfo(on_wait=[w], on_update=[]),
                            )
                        )
                    inst.sync_info = mybir.SyncInfo(
                        on_wait=keep, on_update=list(si.on_update or [])
                    )
                new_insts.append(inst)
            bb.instructions[:] = new_insts


def build_nc(reps=1, dma_only=False, io_bufs=2, xt_bufs=4, pd_bufs=4,
             store_on_act=True, base_eng="sync", inplace_out=False):
    """Build the single-core Bass program (SPMD: same program on all cores).

    reps>1 repeats the whole pipeline (same I/O, idempotent) — used only for
    slope-based device-time measurement in test.py. dma_only strips compute
    (out <- base, xT still loaded) to calibrate the pure DMA roofline.
    """
    import concourse.bass as bass
    import concourse.mybir as mybir
    import concourse.tile as tile

    f32 = mybir.dt.float32
    nc = bass.Bass()
    # xt[half, j, p, s] = x[(half//2)*512 + s, (half%2)*14*128 + j*128 + p]
    xt = nc.dram_tensor("xt", [2 * NMAC, HHALF, 128, 512], f32, kind="ExternalInput")
    base = nc.dram_tensor("base", [S, H], f32, kind="ExternalInput")
    # at[p, j, r] = A_cat^T[j*128 + p, r] (pre-striped on host)
    at = nc.dram_tensor("at", [128, HB, KR], f32, kind="ExternalInput")
    bwt = nc.dram_tensor("bwt", [KR, H], f32, kind="ExternalInput")
    out = nc.dram_tensor("out", [S, H], f32, kind="ExternalOutput")

    # Loads go on the SP HWDGE ring; stores optionally on the ACT ring so a
    # store waiting for compute never head-of-line-blocks the next loads.
    store_eng = nc.scalar if store_on_act else nc.sync
    b_eng = {"sync": nc.sync, "scalar": nc.scalar, "gpsimd": nc.gpsimd}[base_eng]

    with tile.TileContext(nc) as tc:
        with (
            tc.tile_pool(name="const", bufs=1) as const_pool,
            tc.tile_pool(name="xth", bufs=xt_bufs) as xt_pool,
            tc.tile_pool(name="bin", bufs=io_bufs) as b_pool,
            tc.tile_pool(name="oout", bufs=io_bufs) as o_pool,
            tc.tile_pool(name="low", bufs=3) as low_pool,
            tc.tile_pool(name="plow", bufs=2, space="PSUM") as plow_pool,
            tc.tile_pool(name="pd", bufs=pd_bufs, space="PSUM") as pd_pool,
        ):
            at_sb = const_pool.tile([128, HB, KR], f32)
            nc.sync.dma_start(at_sb[:], at[:])
            bwt_sb = const_pool.tile([KR, H], f32)
            nc.sync.dma_start(bwt_sb[:], bwt[:])

            for m in range(NMAC * reps):
                m = m % NMAC
                # xT halves: [128 h-partitions, 14 h-blocks, 512 s]
                halves = []
                for hf in range(2):
                    xh = xt_pool.tile([128, HHALF, 512], f32, tag="xth")
                    nc.sync.dma_start(
                        xh[:], xt[2 * m + hf].rearrange("j p s -> p j s")
                    )
                    halves.append(xh)

                if not dma_only:
                    # down-projection: lowT[kr, s] = sum_h At[h, kr] * xT[h, s]
                    plow = plow_pool.tile([KR, 512], f32, tag="plow")
                    for j in range(HB):
                        nc.tensor.matmul(
                            plow[:],
                            at_sb[:, j, :],
                            halves[j // HHALF][:, j % HHALF, :],
                            start=(j == 0),
                            stop=(j == HB - 1),
                        )
                    lowT = low_pool.tile([KR, 512], f32, tag="lowT")
                    nc.vector.tensor_copy(lowT[:], plow[:])

                for g in range(4):  # 128-row s-blocks within the macro
                    srow = m * 512 + g * S_BLK
                    bt = b_pool.tile([S_BLK, H], f32, tag="base")
                    b_eng.dma_start(bt[:], base[srow : srow + S_BLK, :])
                    if dma_only:
                        store_eng.dma_start(out[srow : srow + S_BLK, :], bt[:])
                        continue
                    # up-projection (K=32, N=512) + base add; optionally add
                    # into the base tile in place (saves an SBUF pool)
                    ot = bt if inplace_out else o_pool.tile(
                        [S_BLK, H], f32, tag="out"
                    )
                    for c in range(HC):
                        pd = pd_pool.tile([S_BLK, 512], f32, tag="pd")
                        nc.tensor.matmul(
                            pd[:],
                            lowT[:, g * S_BLK : (g + 1) * S_BLK],
                            bwt_sb[:, c * 512 : (c + 1) * 512],
                            start=True,
                            stop=True,
                        )
                        nc.vector.tensor_add(
                            ot[:, c * 512 : (c + 1) * 512],
                            pd[:],
                            bt[:, c * 512 : (c + 1) * 512],
                        )
                    store_eng.dma_start(out[srow : srow + S_BLK, :], ot[:])

    _split_sync_waits(nc)
    return nc


def make_in_maps(x, base_output, lora_A, lora_B, top_k_weights, top_k_indices):
    """Host-side prep: expert gather, gate/scaling fold, x h-major relayout."""
    x = np.asarray(x, dtype=np.float32)
    base_output = np.asarray(base_output, dtype=np.float32)
    lora_A = np.asarray(lora_A, dtype=np.float32)
    lora_B = np.asarray(lora_B, dtype=np.float32)
    w = np.asarray(top_k_weights, dtype=np.float32)
    idx = np.asarray(top_k_indices)

    A_sel = lora_A[idx]  # [B, K, R, H]
    At = A_sel.reshape(B, KR, H).transpose(0, 2, 1)  # [B, H, 32]
    # stripe h-major: At_dev[b, p, j, r] = At[b, j*128 + p, r]
    At_dev = np.ascontiguousarray(
        At.reshape(B, HB, 128, KR).transpose(0, 2, 1, 3)
    )  # [B, 128, 28, 32]
    B_sel = lora_B[idx]  # [B, K, H, R]
    Bw = B_sel * (w * SCALING)[:, :, None, None]
    Bwt = np.ascontiguousarray(
        Bw.transpose(0, 1, 3, 2).reshape(B, KR, H)
    )  # [B, 32, H]

    # x -> xt[half, j, p, s]: h-major tiles, each half fully contiguous
    # xt[b, 2m+hf, j, p, s] = x[b, m*512 + s, hf*1792 + j*128 + p]
    xt = np.ascontiguousarray(
        x.reshape(B, NMAC, 512, 2 * HHALF, 128)
        .transpose(0, 1, 3, 4, 2)  # [B, m, jfull, p, s]
        .reshape(B, 2 * NMAC, HHALF, 128, 512)
    )

    return [
        {
            "xt": xt[b],
            "base": np.ascontiguousarray(base_output[b]),
            "at": At_dev[b],
            "bwt": Bwt[b],
        }
        for b in range(B)
    ]


def kernel(x, base_output, lora_A, lora_B, top_k_weights, top_k_indices):
    from concourse.bass_utils import run_bass_kernel_spmd

    nc = _CACHE.get("nc")
    if nc is None:
        nc = build_nc()
        _CACHE["nc"] = nc

    in_maps = make_in_maps(
        x, base_output, lora_A, lora_B, top_k_weights, top_k_indices
    )
    res = run_bass_kernel_spmd(nc, in_maps, list(range(N_CORES)))
    return np.stack([res.results[b]["out"] for b in range(B)], axis=0)
